# revision 1
# baseline (speedup 1.0000x reference)
"""Trainium2 Bass kernel for nn_CollapsedPBFAOptimized (Chebyshev kernelized
linear attention).

Sharding (8 cores): core c handles batch b = c//4 and the 4 heads
[4*(c%4) .. 4*(c%4)+3].  Each core computes a partial output
(x[b] @ w_in_sub -> features -> per-head KV state -> out rows) projected
through its w_out columns; the host sums the 4 partials per batch.

Math: with T_m Chebyshev polynomials and C[m,p] their power-basis
coefficients, sum_m beta_m T_m(q)T_m(k) = sum_{p,r} q^p G[p,r] k^r with
G = C^T diag(beta) C.  The kernel computes power features q^p, k^r on
chip (ACT squares + DVE multiplies), contracts k-powers against v into a
small per-head state KVpow[(r,d),v'], applies C / diag(beta) / C^T to the
state via tiny PE matmuls (d-block-diagonal constant matrices), and
contracts q-powers against the transformed state W[(p,d),v'].
"""
import json
import sys
import numpy as np
from contextlib import ExitStack
from functools import lru_cache

sys.path.insert(0, '/opt/trn_rl_repo')

import concourse.bass as bass
import concourse.tile as tile
from concourse import mybir, bass_utils

# ---------------------------------------------------------------------------
# Toolchain patches
# ---------------------------------------------------------------------------


def _install_patches():
    """This walrus build supports only ONE sync-wait command per instruction.
    (a) Split the TileContext tail drain's waits across multiple Drains.
    (b) Post-process the BIR JSON: hoist excess on_wait entries onto injected
        NoOps on the same engine (engine program order makes this equivalent;
        for queue DMAs the trigger write is the ordering point)."""
    from concourse.tile import ScopedClock
    from concourse import bass2jax

    def _patched_drain_and_barrier(self, tick_clock, wait_clock):
        drain_inst = self.nc.sync.drain()
        wait_clock.add_sem_waits(
            drain_inst.ins, ScopedClock({None: tick_clock.global_clock}))
        si = drain_inst.ins.sync_info
        if si is not None:
            w = list(si.on_wait)
            if len(w) > 1:
                si.on_wait = [w[0]]
                for extra in w[1:]:
                    d2 = self.nc.sync.drain()
                    d2.ins.sync_info = mybir.SyncInfo(on_wait=[extra], on_update=[])
        self.nc.all_engine_barrier()
        assert self.sems is not None
        popped = self.nc._tile_sem_poison_stack.pop()
        assert popped is self._sem_poison
        self.nc.clear_and_free_semaphores(list(self.sems.allocated().values()))
        self.nc.all_engine_barrier()

    tile.TileContext._drain_and_barrier = _patched_drain_and_barrier

    LIMIT = 1

    def split_waits_in_bir_json(bir_json):
        d = json.loads(bir_json.decode() if isinstance(bir_json, bytes) else bir_json)
        for fn in d.get('functions', []):
            for bb in fn.get('blocks', []):
                out, changed = [], False
                for ins in bb.get('instructions', []):
                    si = ins.get('sync_info')
                    waits = (si or {}).get('on_wait') or []
                    if len(waits) > LIMIT:
                        for k, w in enumerate(waits[:-LIMIT]):
                            nop = {'name': ins['name'] + f'-xw{k}',
                                   'engine': ins['engine'], 'opcode': 'NoOp',
                                   'ins': [], 'outs': [],
                                   'sync_info': {'on_wait': [w], 'on_update': []}}
                            if 'debug' in ins:
                                nop['debug'] = ins['debug']
                            out.append(nop)
                        si['on_wait'] = waits[-LIMIT:]
                        changed = True
                    out.append(ins)
                if changed:
                    bb['instructions'] = out
        return json.dumps(d).encode()

    if not getattr(bass_utils.compile_bir_kernel, '_wait_patched', False):
        orig = bass_utils.compile_bir_kernel

        def patched(bir_json, tmpdir, neff_name='file.neff'):
            return orig(split_waits_in_bir_json(bir_json), tmpdir, neff_name)

        patched._wait_patched = True
        bass_utils.compile_bir_kernel = patched
        bass2jax.compile_bir_kernel = patched


_install_patches()

# ---------------------------------------------------------------------------
# Problem constants (hardcoded per the task contract)
# ---------------------------------------------------------------------------
B, S, D = 2, 4096, 1024
H, DH = 16, 64
NP = 11                      # Chebyshev orders / power degrees 0..10
SCALE = DH ** -0.5
HPC = 4                      # heads per core
NCORES = 8
F32 = mybir.dt.float32

# block index lists for the two state transforms (i = out chunk, j = in chunk)
C1_BLOCKS = [(i, j) for i in range(6) for j in range(i + 1)]       # U = C . KVpow
C2_BLOCKS = [(i, j) for i in range(6) for j in range(i, 6)]        # W = C^T . (beta*U)


def _cheb_C():
    C = np.zeros((NP, NP), dtype=np.float64)
    for m in range(NP):
        e = np.zeros(m + 1)
        e[m] = 1.0
        c = np.polynomial.chebyshev.cheb2poly(e) if m > 0 else np.array([1.0])
        C[m, :len(c)] = c
    return C


def _chunk_w(i):
    return 128 if i < 5 else 64


# ---------------------------------------------------------------------------
# Device program
# ---------------------------------------------------------------------------


def _build_program():
    nc = bass.Bass('TRN2', target_bir_lowering=False, debug=False,
                   num_devices=NCORES)
    ap = {}
    ap['xT'] = nc.dram_tensor('xT', (D, S), F32, kind='ExternalInput').ap()
    ap['wqT'] = nc.dram_tensor('wqT', (D, 256), F32, kind='ExternalInput').ap()
    ap['wkvT'] = nc.dram_tensor('wkvT', (D, 512), F32, kind='ExternalInput').ap()
    ap['woT'] = nc.dram_tensor('woT', (256, D), F32, kind='ExternalInput').ap()
    ap['c1t'] = nc.dram_tensor('c1t', (len(C1_BLOCKS), 128, 128), F32, kind='ExternalInput').ap()
    ap['c2t'] = nc.dram_tensor('c2t', (len(C2_BLOCKS), 128, 128), F32, kind='ExternalInput').ap()
    ap['betac'] = nc.dram_tensor('betac', (128, 24), F32, kind='ExternalInput').ap()
    ap['eye'] = nc.dram_tensor('eye', (128, 128), F32, kind='ExternalInput').ap()
    ap['outp'] = nc.dram_tensor('outp', (D, S), F32, kind='ExternalOutput').ap()
    import os
    ap['_debug'] = os.environ.get('KBDBG', '') == '1'
    if ap['_debug']:
        ap['dbg_qb'] = nc.dram_tensor('dbg_qb', (2, 128, S), F32, kind='ExternalOutput').ap()
        ap['dbg_k'] = nc.dram_tensor('dbg_k', (128, 8192), F32, kind='ExternalOutput').ap()
        ap['dbg_v'] = nc.dram_tensor('dbg_v', (128, 8192), F32, kind='ExternalOutput').ap()
        ap['dbg_st'] = nc.dram_tensor('dbg_st', (4, 64, 704), F32, kind='ExternalOutput').ap()
        ap['dbg_wd'] = nc.dram_tensor('dbg_wd', (4, 128, 384), F32, kind='ExternalOutput').ap()
        ap['dbg_oT'] = nc.dram_tensor('dbg_oT', (2, 128, S), F32, kind='ExternalOutput').ap()

    with tile.TileContext(nc) as tc:
        with ExitStack() as ctx:
            _emit(nc, tc, ctx, ap)
    return nc


def _emit(nc, tc, ctx, ap):
    TS = mybir.AluOpType  # alu ops

    const = ctx.enter_context(tc.tile_pool(name='const', bufs=1))
    persist = ctx.enter_context(tc.tile_pool(name='persist', bufs=1))
    kv_ctx = ExitStack()
    kv_pool = kv_ctx.enter_context(tc.tile_pool(name='kv', bufs=1))

    eye_sb = const.tile([128, 128], F32, tag='eye', name='eye')
    nc.sync.dma_start(eye_sb[:], ap['eye'][:])
    ones_sb = const.tile([128, 512], F32, tag='ones', name='ones')
    nc.gpsimd.memset(ones_sb[:], 1.0)
    betac_sb = const.tile([128, 24], F32, tag='betac', name='betac')
    nc.sync.dma_start(betac_sb[:], ap['betac'][:])

    k_all = kv_pool.tile([128, 32 * 256], F32, tag='k_all', name='k_all')   # [s%128, chunk*256 + h*64 + d]
    v_all = kv_pool.tile([128, 32 * 256], F32, tag='v_all', name='v_all')
    qb = [persist.tile([128, S], F32, tag=f'qb{hp}', name=f'qb{hp}') for hp in range(2)]

    # ---------------- Phase 1: fused QKV projection -----------------------
    with tc.tile_pool(name='ph1w', bufs=1) as ph1w, \
         tc.tile_pool(name='xt', bufs=2) as xtp, \
         tc.tile_pool(name='ps1', bufs=2, space='PSUM') as ps1:
        wq_sb = []
        wkv_sb = []
        for i in range(8):
            wq = ph1w.tile([128, 256], F32, tag=f'wq{i}', name=f'wq{i}')
            nc.sync.dma_start(wq[:], ap['wqT'][i * 128:(i + 1) * 128, :])
            wq_sb.append(wq)
            wkv = ph1w.tile([128, 512], F32, tag=f'wkv{i}', name=f'wkv{i}')
            nc.sync.dma_start(wkv[:], ap['wkvT'][i * 128:(i + 1) * 128, :])
            wkv_sb.append(wkv)

        for ss in range(8):
            xt = []
            for i in range(8):
                t = xtp.tile([128, 512], F32, tag=f'xt{i}', name=f'xt{i}')
                nc.sync.dma_start(t[:], ap['xT'][i * 128:(i + 1) * 128,
                                                 ss * 512:(ss + 1) * 512])
                xt.append(t)
            for hp in range(2):
                pq = ps1.tile([128, 512], F32, tag='pq', name='pq')
                for i in range(8):
                    nc.tensor.matmul(pq[:], wq_sb[i][:, hp * 128:(hp + 1) * 128],
                                     xt[i][:], start=(i == 0), stop=(i == 7))
                nc.vector.tensor_scalar(qb[hp][:, ss * 512:(ss + 1) * 512], pq[:],
                                        -1.0, 1.0, op0=TS.max, op1=TS.min)
            for sc in range(4):
                pkv = ps1.tile([128, 512], F32, tag='pkv', name='pkv')
                for i in range(8):
                    nc.tensor.matmul(pkv[:], xt[i][:, sc * 128:(sc + 1) * 128],
                                     wkv_sb[i][:], start=(i == 0), stop=(i == 7))
                ch = ss * 4 + sc
                nc.vector.tensor_scalar(k_all[:, ch * 256:(ch + 1) * 256],
                                        pkv[:, 0:256], -1.0, 1.0,
                                        op0=TS.max, op1=TS.min)
                nc.scalar.copy(v_all[:, ch * 256:(ch + 1) * 256], pkv[:, 256:512])

    if ap['_debug']:
        nc.sync.dma_start(ap['dbg_k'][:], k_all[:])
        nc.sync.dma_start(ap['dbg_v'][:], v_all[:])
        for hp in range(2):
            nc.sync.dma_start(ap['dbg_qb'][hp], qb[hp][:])

    # ---------------- Phase 2: k-powers + per-head KV state ---------------
    # NB: a matmul's start=True clears its whole PSUM *bank*, so every
    # accumulation group must own its bank exclusively.
    # KP holds k^1..k^10 (10 blocks of 512); psum pst: bank0 = r1..r8,
    # bank1 cols 512:640 = r9..r10; pr0 (own bank) = r0 (= colsum of v).
    state_sb = []
    with tc.tile_pool(name='kp', bufs=2) as kpp, \
         tc.tile_pool(name='ps2', bufs=2, space='PSUM') as ps2:
        for h in range(HPC):
            pst = ps2.tile([64, 1024], F32, tag='pst', name='pst')
            pr0 = ps2.tile([64, 64], F32, tag='pr0', name='pr0')
            for g in range(4):
                KP = kpp.tile([128, 10 * 512], F32, tag='kp', name='kp')  # (r-1)*512 + cl*64 + d
                k3 = k_all[:].rearrange("p (c w) -> p c w", w=256)[
                    :, 8 * g:8 * (g + 1), h * 64:(h + 1) * 64]     # [128, 8, 64]

                def kp3(p):
                    return KP[:, (p - 1) * 512:p * 512].rearrange(
                        "p (c d) -> p c d", d=64)

                SQ = mybir.ActivationFunctionType.Square
                nc.scalar.copy(kp3(1), k3)                                # k^1
                nc.scalar.activation(kp3(2), k3, SQ)                      # k^2
                nc.vector.tensor_tensor(kp3(3), kp3(2), k3, op=TS.mult)   # k^3
                nc.scalar.activation(kp3(4), kp3(2), SQ)                  # k^4
                nc.vector.tensor_tensor(kp3(5), kp3(4), k3, op=TS.mult)   # k^5
                nc.scalar.activation(kp3(6), kp3(3), SQ)                  # k^6
                nc.vector.tensor_tensor(kp3(7), kp3(3), kp3(4), op=TS.mult)
                nc.scalar.activation(kp3(8), kp3(4), SQ)                  # k^8
                nc.vector.tensor_tensor(kp3(9), kp3(4), kp3(5), op=TS.mult)
                nc.scalar.activation(kp3(10), kp3(5), SQ)                 # k^10

                for cl in range(8):
                    c = 8 * g + cl
                    st, sp = (c == 0), (c == 31)
                    vsl = v_all[:, c * 256 + h * 64:c * 256 + (h + 1) * 64]
                    r18 = KP[:].rearrange("p (r w) -> p r w", w=512)[
                        :, 0:8, cl * 64:(cl + 1) * 64]
                    r910 = KP[:].rearrange("p (r w) -> p r w", w=512)[
                        :, 8:10, cl * 64:(cl + 1) * 64]
                    nc.tensor.matmul(pst[:, 0:512], vsl, r18, start=st, stop=sp)
                    nc.tensor.matmul(pst[:, 512:640], vsl, r910, start=st, stop=sp)
                    nc.tensor.matmul(pr0[:], vsl, ones_sb[:, 0:64],
                                     start=st, stop=sp)
            ssb = persist.tile([64, 704], F32, tag=f'st{h}', name=f'st{h}')
            nc.scalar.copy(ssb[:, 0:64], pr0[:])
            nc.scalar.copy(ssb[:, 64:576], pst[:, 0:512])
            nc.scalar.copy(ssb[:, 576:704], pst[:, 512:640])
            state_sb.append(ssb)
    if ap['_debug']:
        for h in range(HPC):
            nc.sync.dma_start(ap['dbg_st'][h], state_sb[h][:])
    kv_ctx.close()

    # ---------------- Phase 3: state transforms ---------------------------
    wdup, wdup2 = [], []
    with tc.tile_pool(name='c1p', bufs=2) as c1p, \
         tc.tile_pool(name='stT', bufs=1) as stTp, \
         tc.tile_pool(name='usc', bufs=1) as uscp:
        # transpose state -> stateT chunks [(r,d), v']
        stT = [[None] * 6 for _ in range(HPC)]
        with tc.tile_pool(name='psT', bufs=2, space='PSUM') as psTp:
            for h in range(HPC):
                for j in range(6):
                    w = _chunk_w(j)
                    pT = psTp.tile([128, 64], F32, tag='pT', name='pT')
                    nc.tensor.transpose(pT[0:w, :],
                                        state_sb[h][:, j * 128:j * 128 + w],
                                        eye_sb[0:64, 0:64])
                    t = stTp.tile([128, 64], F32, tag=f'stT{h}_{j}', name=f'stT{h}_{j}')
                    nc.scalar.copy(t[0:w, :], pT[0:w, :])
                    stT[h][j] = t
        # U = C . KVpow (per d), then beta scale on eviction
        usc = [[None] * 6 for _ in range(HPC)]
        psU = {}
        with tc.tile_pool(name='psU', bufs=1, space='PSUM') as psUp:
            for idx, (i, j) in enumerate(C1_BLOCKS):
                iw, jw = _chunk_w(i), _chunk_w(j)
                c1 = c1p.tile([128, 128], F32, tag='c1', name='c1')
                nc.sync.dma_start(c1[:], ap['c1t'][idx])
                for h in range(HPC):
                    if j == 0:
                        psU[h] = psUp.tile([128, 64], F32, tag=f'pU{h}', name=f'pU{h}')
                    nc.tensor.matmul(psU[h][0:iw, :], c1[0:jw, 0:iw],
                                     stT[h][j][0:jw, :], start=(j == 0), stop=(j == i))
                    if j == i:
                        t = uscp.tile([128, 64], F32, tag=f'usc{h}_{i}', name=f'usc{h}_{i}')
                        nc.vector.tensor_scalar(
                            t[0:iw, :], psU[h][0:iw, :],
                            betac_sb[0:iw, h * 6 + i:h * 6 + i + 1], None,
                            op0=TS.mult)
                        usc[h][i] = t
        # W = C^T . (beta*U); evict into Wdup, then build the swapped copy
        for h in range(HPC):
            wdup.append(persist.tile([128, 6 * 64], F32, tag=f'wd{h}', name=f'wd{h}'))
            wdup2.append(persist.tile([128, 6 * 64], F32, tag=f'wd2{h}', name=f'wd2{h}'))
        psW = {}
        with tc.tile_pool(name='psW', bufs=1, space='PSUM') as psWp:
            for idx, (i, j) in enumerate(C2_BLOCKS):
                iw, jw = _chunk_w(i), _chunk_w(j)
                c2 = c1p.tile([128, 128], F32, tag='c1', name='c1')
                nc.sync.dma_start(c2[:], ap['c2t'][idx])
                for h in range(HPC):
                    if j == i:
                        psW[h] = psWp.tile([128, 64], F32, tag=f'pW{h}', name=f'pW{h}')
                    nc.tensor.matmul(psW[h][0:iw, :], c2[0:jw, 0:iw],
                                     usc[h][j][0:jw, :], start=(j == i), stop=(j == 5))
                    if j == 5:
                        nc.scalar.copy(wdup[h][0:iw, i * 64:(i + 1) * 64],
                                       psW[h][0:iw, :])
        for h in range(HPC):
            nc.vector.tensor_copy(wdup2[h][64:128, :], wdup[h][0:64, :])
            nc.vector.tensor_copy(wdup2[h][0:64, :], wdup[h][64:128, :])

    if ap['_debug']:
        for h in range(HPC):
            nc.sync.dma_start(ap['dbg_wd'][h], wdup[h][:])

    # ---------------- Phase 4a: q-powers + q-side einsum ------------------
    ph4_pool = ctx.enter_context(tc.tile_pool(name='ph4', bufs=1))
    outT = [ph4_pool.tile([128, S], F32, tag=f'outT{hp}', name=f'outT{hp}') for hp in range(2)]
    with tc.tile_pool(name='qp', bufs=2) as qpp, \
         tc.tile_pool(name='ps4', bufs=2, space='PSUM') as ps4:
        for hp in range(2):
            for t in range(8):
                qsl = qb[hp][:, t * 512:(t + 1) * 512]
                QP = {p: qpp.tile([128, 512], F32, tag=f'qp{p}', name=f'qp{p}')
                      for p in range(2, 11)}
                SQ = mybir.ActivationFunctionType.Square
                nc.scalar.activation(QP[2][:], qsl, SQ)
                nc.vector.tensor_tensor(QP[3][:], QP[2][:], qsl, op=TS.mult)
                nc.scalar.activation(QP[4][:], QP[2][:], SQ)
                nc.vector.tensor_tensor(QP[5][:], QP[4][:], qsl, op=TS.mult)
                nc.scalar.activation(QP[6][:], QP[3][:], SQ)
                nc.vector.tensor_tensor(QP[7][:], QP[3][:], QP[4][:], op=TS.mult)
                nc.scalar.activation(QP[8][:], QP[4][:], SQ)
                nc.vector.tensor_tensor(QP[9][:], QP[4][:], QP[5][:], op=TS.mult)
                nc.scalar.activation(QP[10][:], QP[5][:], SQ)
                for role in range(2):
                    h = 2 * hp + role
                    lo, hi = role * 64, (role + 1) * 64
                    pO = ps4.tile([64, 512], F32, tag=f'pO{role}', name=f'pO{role}')
                    for p in range(NP):
                        i, par = p // 2, p % 2
                        if role == 0:
                            wsrc = wdup[h] if par == 0 else wdup2[h]
                            lhsT = wsrc[0:64, i * 64:(i + 1) * 64]
                        else:
                            wsrc = wdup2[h] if par == 0 else wdup[h]
                            lhsT = wsrc[64:128, i * 64:(i + 1) * 64]
                        if p == 0:
                            rhs = ones_sb[lo:hi, 0:512]
                        elif p == 1:
                            rhs = qsl[lo:hi, :]
                        else:
                            rhs = QP[p][lo:hi, :]
                        nc.tensor.matmul(pO[:], lhsT, rhs,
                                         start=(p == 0), stop=(p == NP - 1))
                    dst = outT[hp][lo:hi, t * 512:(t + 1) * 512]
                    if role == 0:
                        nc.scalar.copy(dst, pO[:])
                    else:
                        nc.vector.tensor_copy(dst, pO[:])

    if ap['_debug']:
        for hp in range(2):
            nc.sync.dma_start(ap['dbg_oT'][hp], outT[hp][:])

    # ---------------- Phase 4b: output projection -------------------------
    with tc.tile_pool(name='ph4w', bufs=1) as ph4w, \
         tc.tile_pool(name='osb', bufs=4) as osbp, \
         tc.tile_pool(name='ps5', bufs=4, space='PSUM') as ps5:
        woT_sb = []
        for kcI in range(2):
            w = ph4w.tile([128, D], F32, tag=f'wo{kcI}', name=f'wo{kcI}')
            nc.sync.dma_start(w[:], ap['woT'][kcI * 128:(kcI + 1) * 128, :])
            woT_sb.append(w)
        for o in range(8):
            for s8 in range(8):
                pP = ps5.tile([128, 512], F32, tag='pP', name='pP')
                nc.tensor.matmul(pP[:], woT_sb[0][:, o * 128:(o + 1) * 128],
                                 outT[0][:, s8 * 512:(s8 + 1) * 512],
                                 start=True, stop=False)
                nc.tensor.matmul(pP[:], woT_sb[1][:, o * 128:(o + 1) * 128],
                                 outT[1][:, s8 * 512:(s8 + 1) * 512],
                                 start=False, stop=True)
                ob = osbp.tile([128, 512], F32, tag='ob', name='ob')
                if (o + s8) % 2 == 0:
                    nc.scalar.copy(ob[:], pP[:])
                else:
                    nc.vector.tensor_copy(ob[:], pP[:])
                nc.sync.dma_start(ap['outp'][o * 128:(o + 1) * 128,
                                             s8 * 512:(s8 + 1) * 512], ob[:])


@lru_cache(maxsize=1)
def _get_program():
    return _build_program()


# ---------------------------------------------------------------------------
# Host-side constants and per-core input packing
# ---------------------------------------------------------------------------


@lru_cache(maxsize=1)
def _host_consts():
    C = _cheb_C()
    n1 = len(C1_BLOCKS)
    c1t = np.zeros((n1, 128, 128), dtype=np.float32)
    dd = np.eye(64, dtype=np.float64)
    for idx, (i, j) in enumerate(C1_BLOCKS):
        for ml in range(2):
            m = 2 * i + ml
            if m >= NP:
                continue
            for rl in range(2):
                r = 2 * j + rl
                if r >= NP:
                    continue
                c1t[idx, rl * 64:(rl + 1) * 64, ml * 64:(ml + 1) * 64] = C[m, r] * dd
    n2 = len(C2_BLOCKS)
    c2t = np.zeros((n2, 128, 128), dtype=np.float32)
    for idx, (i, j) in enumerate(C2_BLOCKS):
        for pl in range(2):
            p = 2 * i + pl
            if p >= NP:
                continue
            for ml in range(2):
                m = 2 * j + ml
                if m >= NP:
                    continue
                c2t[idx, ml * 64:(ml + 1) * 64, pl * 64:(pl + 1) * 64] = C[m, p] * dd
    eye = np.eye(128, dtype=np.float32)
    return c1t, c2t, eye


last_results = None


def kernel(x, w_in, w_out, beta):
    x = np.asarray(x, dtype=np.float32)
    w_in = np.asarray(w_in, dtype=np.float32)
    w_out = np.asarray(w_out, dtype=np.float32)
    beta = np.asarray(beta, dtype=np.float32)
    nc = _get_program()
    c1t, c2t, eye = _host_consts()

    xT = [np.ascontiguousarray(x[b].T) for b in range(B)]
    in_maps = []
    for c in range(NCORES):
        b, hg = c // 4, c % 4
        heads = [4 * hg + j for j in range(HPC)]
        wqT = np.empty((D, 256), dtype=np.float32)
        wkvT = np.empty((D, 512), dtype=np.float32)
        for hl, h in enumerate(heads):
            wqT[:, hl * 64:(hl + 1) * 64] = (SCALE * w_in[h * DH:(h + 1) * DH, :]).T
            wkvT[:, hl * 64:(hl + 1) * 64] = (SCALE * w_in[D + h * DH:D + (h + 1) * DH, :]).T
            wkvT[:, 256 + hl * 64:256 + (hl + 1) * 64] = w_in[2 * D + h * DH:2 * D + (h + 1) * DH, :].T
        woT = np.empty((256, D), dtype=np.float32)
        for hl, h in enumerate(heads):
            woT[hl * 64:(hl + 1) * 64, :] = w_out[:, h * DH:(h + 1) * DH].T
        betac = np.zeros((128, 24), dtype=np.float32)
        for hl, h in enumerate(heads):
            for i in range(6):
                for ml in range(2):
                    m = 2 * i + ml
                    if m < NP:
                        betac[ml * 64:(ml + 1) * 64, hl * 6 + i] = beta[h, m]
        in_maps.append({
            'xT': np.ascontiguousarray(xT[b]),
            'wqT': np.ascontiguousarray(wqT),
            'wkvT': np.ascontiguousarray(wkvT),
            'woT': np.ascontiguousarray(woT),
            'c1t': c1t, 'c2t': c2t, 'betac': betac, 'eye': eye,
        })

    res = bass_utils.run_bass_kernel_spmd(nc, in_maps, core_ids=list(range(NCORES)))
    global last_results
    last_results = res

    out = np.zeros((B, S, D), dtype=np.float32)
    for c in range(NCORES):
        out[c // 4] += res.results[c]['outp'].T
    return out



# revision 2
# speedup vs baseline: 3.9738x; 3.9738x over previous
"""Trainium2 Bass kernel for nn_CollapsedPBFAOptimized (Chebyshev kernelized
linear attention), bf16 fast path.

Sharding (8 cores): core c handles batch b = c//4 and the 4 heads
[4*(c%4) .. 4*(c%4)+3].  Each core computes a partial output
(x[b] @ w_in_sub -> features -> per-head KV state -> out rows) projected
through its w_out columns; the host sums the 4 partials per batch.

Math: the collapsed beta is zero for m=0 and m>=6, so the kernel is
  out_i = sum_{m=1..5} beta_m <T_m(q_i), T_m(k_j)> v_j
with only FIVE Chebyshev features per dim.  Features are computed with
scaled one-op recurrences (f1=t, f2=t^2-1/2, f3=(t^2-3/4)t, f4=f2^2-1/8,
f5=f2*f3-t/16 => f_m = T_m/2^(m-1)), and the scale correction
beta_m * 4^(m-1) is folded into the per-head state when assembling the
block-diagonal weight tiles for the query-side contraction.

All matmuls run in bf16 (1 PE cycle/row vs 4 for fp32); PSUM accumulates
fp32.  Verified end-to-end rel err ~5e-3 vs the fp32 reference.
"""
import json
import sys
import numpy as np
from contextlib import ExitStack
from functools import lru_cache

sys.path.insert(0, '/opt/trn_rl_repo')

import concourse.bass as bass
import concourse.tile as tile
from concourse import mybir, bass_utils

# ---------------------------------------------------------------------------
# Toolchain patches (walrus on this image supports one sync-wait per inst)
# ---------------------------------------------------------------------------


def _install_patches():
    from concourse.tile import ScopedClock
    from concourse import bass2jax

    def _patched_drain_and_barrier(self, tick_clock, wait_clock):
        drain_inst = self.nc.sync.drain()
        wait_clock.add_sem_waits(
            drain_inst.ins, ScopedClock({None: tick_clock.global_clock}))
        si = drain_inst.ins.sync_info
        if si is not None:
            w = list(si.on_wait)
            if len(w) > 1:
                si.on_wait = [w[0]]
                for extra in w[1:]:
                    d2 = self.nc.sync.drain()
                    d2.ins.sync_info = mybir.SyncInfo(on_wait=[extra], on_update=[])
        self.nc.all_engine_barrier()
        assert self.sems is not None
        popped = self.nc._tile_sem_poison_stack.pop()
        assert popped is self._sem_poison
        self.nc.clear_and_free_semaphores(list(self.sems.allocated().values()))
        self.nc.all_engine_barrier()

    tile.TileContext._drain_and_barrier = _patched_drain_and_barrier

    LIMIT = 1

    def split_waits_in_bir_json(bir_json):
        d = json.loads(bir_json.decode() if isinstance(bir_json, bytes) else bir_json)
        for fn in d.get('functions', []):
            for bb in fn.get('blocks', []):
                out, changed = [], False
                for ins in bb.get('instructions', []):
                    si = ins.get('sync_info')
                    waits = (si or {}).get('on_wait') or []
                    if len(waits) > LIMIT:
                        for k, w in enumerate(waits[:-LIMIT]):
                            nop = {'name': ins['name'] + f'-xw{k}',
                                   'engine': ins['engine'], 'opcode': 'NoOp',
                                   'ins': [], 'outs': [],
                                   'sync_info': {'on_wait': [w], 'on_update': []}}
                            if 'debug' in ins:
                                nop['debug'] = ins['debug']
                            out.append(nop)
                        si['on_wait'] = waits[-LIMIT:]
                        changed = True
                    out.append(ins)
                if changed:
                    bb['instructions'] = out
        return json.dumps(d).encode()

    if not getattr(bass_utils.compile_bir_kernel, '_wait_patched', False):
        orig = bass_utils.compile_bir_kernel

        def patched(bir_json, tmpdir, neff_name='file.neff'):
            return orig(split_waits_in_bir_json(bir_json), tmpdir, neff_name)

        patched._wait_patched = True
        bass_utils.compile_bir_kernel = patched
        bass2jax.compile_bir_kernel = patched


_install_patches()

# ---------------------------------------------------------------------------
# Problem constants (hardcoded per the task contract)
# ---------------------------------------------------------------------------
B, S, D = 2, 4096, 1024
H, DH = 16, 64
NF = 5                       # Chebyshev features T_1..T_5 (beta_0 = beta_{6..} = 0)
SCALE = DH ** -0.5
HPC = 4                      # heads per core
NCORES = 8
F32 = mybir.dt.float32
BF16 = mybir.dt.bfloat16
NCHUNK = S // 128            # 32 seq chunks of 128
FTC = HPC * NF * 64          # FT columns per chunk = 1280

# ---------------------------------------------------------------------------
# Device program
# ---------------------------------------------------------------------------


def _build_program():
    nc = bass.Bass('TRN2', target_bir_lowering=False, debug=False,
                   num_devices=NCORES)
    ap = {}
    ap['xT'] = nc.dram_tensor('xT', (D, S), BF16, kind='ExternalInput').ap()
    ap['wqT'] = nc.dram_tensor('wqT', (D, 256), BF16, kind='ExternalInput').ap()
    ap['wkvT'] = nc.dram_tensor('wkvT', (D, 512), BF16, kind='ExternalInput').ap()
    ap['woT'] = nc.dram_tensor('woT', (256, D), BF16, kind='ExternalInput').ap()
    ap['betac'] = nc.dram_tensor('betac', (128, 20), F32, kind='ExternalInput').ap()
    ap['outp'] = nc.dram_tensor('outp', (D, S), BF16, kind='ExternalOutput').ap()
    import os
    ap['_debug'] = os.environ.get('KBDBG', '') == '1'
    if ap['_debug']:
        ap['dbg_qb'] = nc.dram_tensor('dbg_qb', (2, 128, S), BF16, kind='ExternalOutput').ap()
        ap['dbg_ft'] = nc.dram_tensor('dbg_ft', (128, 4 * FTC), BF16, kind='ExternalOutput').ap()
        ap['dbg_v'] = nc.dram_tensor('dbg_v', (128, 8192), BF16, kind='ExternalOutput').ap()
        ap['dbg_st'] = nc.dram_tensor('dbg_st', (4, 128, 192), BF16, kind='ExternalOutput').ap()
        ap['dbg_wm'] = nc.dram_tensor('dbg_wm', (10, 128, 128), BF16, kind='ExternalOutput').ap()
        ap['dbg_oT'] = nc.dram_tensor('dbg_oT', (2, 128, S), BF16, kind='ExternalOutput').ap()

    with tile.TileContext(nc) as tc:
        with ExitStack() as ctx:
            _emit(nc, tc, ctx, ap)
    return nc


def _emit(nc, tc, ctx, ap):
    TS = mybir.AluOpType
    SQ = mybir.ActivationFunctionType.Square

    const = ctx.enter_context(tc.tile_pool(name='const', bufs=1))
    persist = ctx.enter_context(tc.tile_pool(name='persist', bufs=1))

    betac_sb = const.tile([128, 20], F32, tag='betac', name='betac')
    nc.sync.dma_start(betac_sb[:], ap['betac'][:])

    qb = [persist.tile([128, S], BF16, tag=f'qb{hp}', name=f'qb{hp}') for hp in range(2)]
    FT = persist.tile([128, NCHUNK * FTC], BF16, tag='FT', name='FT')
    v_all = persist.tile([128, NCHUNK * 256], BF16, tag='v_all', name='v_all')

    # FT column layout: c*1280 + hl*320 + m*64 + d   (m = 0..4 <-> T_1..T_5)
    ftv = FT[:].rearrange("p (c h m d) -> p c h m d", h=HPC, m=NF, d=64)

    # ---------------- Phase 1: fused QKV projection + k/q features --------
    with tc.tile_pool(name='ph1w', bufs=1) as ph1w, \
         tc.tile_pool(name='xt', bufs=2) as xtp, \
         tc.tile_pool(name='fsc', bufs=2) as fscp, \
         tc.tile_pool(name='ps1', bufs=2, space='PSUM') as ps1:
        wq_sb = []
        wkv_sb = []
        for i in range(8):
            wq = ph1w.tile([128, 256], BF16, tag=f'wq{i}', name=f'wq{i}')
            nc.sync.dma_start(wq[:], ap['wqT'][i * 128:(i + 1) * 128, :])
            wq_sb.append(wq)
            wkv = ph1w.tile([128, 512], BF16, tag=f'wkv{i}', name=f'wkv{i}')
            nc.sync.dma_start(wkv[:], ap['wkvT'][i * 128:(i + 1) * 128, :])
            wkv_sb.append(wkv)

        for ss in range(8):
            xt = []
            for i in range(8):
                t = xtp.tile([128, 512], BF16, tag=f'xt{i}', name=f'xt{i}')
                nc.sync.dma_start(t[:], ap['xT'][i * 128:(i + 1) * 128,
                                                 ss * 512:(ss + 1) * 512])
                xt.append(t)
            for hp in range(2):
                pq = ps1.tile([128, 512], F32, tag='pq', name='pq')
                for i in range(8):
                    nc.tensor.matmul(pq[:], wq_sb[i][:, hp * 128:(hp + 1) * 128],
                                     xt[i][:], start=(i == 0), stop=(i == 7))
                nc.vector.tensor_scalar(qb[hp][:, ss * 512:(ss + 1) * 512], pq[:],
                                        -1.0, 1.0, op0=TS.max, op1=TS.min)
            for sc in range(4):
                pkv = ps1.tile([128, 512], F32, tag='pkv', name='pkv')
                for i in range(8):
                    nc.tensor.matmul(pkv[:], xt[i][:, sc * 128:(sc + 1) * 128],
                                     wkv_sb[i][:], start=(i == 0), stop=(i == 7))
                c = ss * 4 + sc
                # k (clamped) -> FT m=0 slots, strided over heads
                nc.vector.tensor_scalar(ftv[:, c, :, 0, :], pkv[:, 0:256],
                                        -1.0, 1.0, op0=TS.max, op1=TS.min)
                nc.scalar.copy(v_all[:, c * 256:(c + 1) * 256], pkv[:, 256:512])
            # features for this ss's 4 chunks, in 2 chunk-pairs (FD=512 ops)
            for gp in range(2):
                c0 = ss * 4 + 2 * gp
                f1 = ftv[:, c0:c0 + 2, :, 0, :]
                f2 = ftv[:, c0:c0 + 2, :, 1, :]
                f3 = ftv[:, c0:c0 + 2, :, 2, :]
                f4 = ftv[:, c0:c0 + 2, :, 3, :]
                f5 = ftv[:, c0:c0 + 2, :, 4, :]
                hsq = fscp.tile([128, 512], BF16, tag='hsq', name='hsq')
                s4t = fscp.tile([128, 512], BF16, tag='s4t', name='s4t')
                t5t = fscp.tile([128, 512], BF16, tag='t5t', name='t5t')
                nc.scalar.activation(hsq[:], f1, SQ)
                nc.vector.tensor_scalar(f2, hsq[:], -0.5, None, op0=TS.add)
                nc.vector.scalar_tensor_tensor(f3, hsq[:], -0.75, f1,
                                               op0=TS.add, op1=TS.mult)
                nc.scalar.activation(s4t[:], f2, SQ)
                nc.vector.tensor_scalar(f4, s4t[:], -0.125, None, op0=TS.add)
                nc.vector.tensor_tensor(t5t[:], f2, f3, op=TS.mult)
                nc.vector.scalar_tensor_tensor(f5, f1, -0.0625, t5t[:],
                                               op0=TS.mult, op1=TS.add)

    if ap['_debug']:
        nc.sync.dma_start(ap['dbg_ft'][:], FT[:, 0:4 * FTC])
        nc.sync.dma_start(ap['dbg_v'][:], v_all[:])
        for hp in range(2):
            nc.sync.dma_start(ap['dbg_qb'][hp], qb[hp][:])

    # ---------------- Phase 2: per-head KV state (transposed) -------------
    # stT[h] psum [128, 192]: cols j*64:(j+1)*64 = v', rows = (m,d) pairs
    # (j=0: m=1,2; j=1: m=3,4; j=2: m=5 rows 0:64).  One bank per head; the
    # j groups share it (start=True clears the whole bank -> later groups
    # overwrite-on-first-touch).
    stT_sb = [persist.tile([128, 192], BF16, tag=f'stT{h}', name=f'stT{h}')
              for h in range(HPC)]
    with tc.tile_pool(name='ps2', bufs=1, space='PSUM') as ps2:
        stp = [ps2.tile([128, 192], F32, tag=f'stp{h}', name=f'stp{h}')
               for h in range(HPC)]
        for c in range(NCHUNK):
            for h in range(HPC):
                base = c * FTC + h * 320
                vsl = v_all[:, c * 256 + h * 64:c * 256 + (h + 1) * 64]
                for j in range(3):
                    w = 128 if j < 2 else 64
                    lhsT = FT[:, base + j * 128: base + j * 128 + w]
                    nc.tensor.matmul(stp[h][0:w, j * 64:(j + 1) * 64], lhsT, vsl,
                                     start=(c == 0 and j == 0),
                                     stop=(c == NCHUNK - 1 and j == 2),
                                     skip_group_check=(not (c == 0 and j == 0)
                                                      and not (c == NCHUNK - 1 and j == 2)))
        for h in range(HPC):
            if h % 2 == 0:
                nc.scalar.copy(stT_sb[h][:], stp[h][:])
            else:
                nc.vector.tensor_copy(stT_sb[h][:], stp[h][:])

    if ap['_debug']:
        for h in range(HPC):
            nc.sync.dma_start(ap['dbg_st'][h], stT_sb[h][:])

    # ---------------- Phase 3: block-diag weight tiles --------------------
    # Wm[hp][m] [128,128] bf16: rows 0:64 = beta'_{hA,m} * stT_hA(m),
    # rows 64:128 cols 64:128 = head B.  beta' = beta_m * 4^(m-1).
    Wm = [[persist.tile([128, 128], BF16, tag=f'wm{hp}_{m}', name=f'wm{hp}_{m}')
           for m in range(NF)] for hp in range(2)]
    for hp in range(2):
        for m in range(NF):
            nc.gpsimd.memset(Wm[hp][m][:], 0.0)
            j, ro = m // 2, (m % 2) * 64
            nc.vector.tensor_scalar(
                Wm[hp][m][0:64, 0:64],
                stT_sb[2 * hp][ro:ro + 64, j * 64:(j + 1) * 64],
                betac_sb[ro:ro + 64, hp * 10 + m:hp * 10 + m + 1], None,
                op0=TS.mult)
            nc.vector.tensor_scalar(
                Wm[hp][m][64:128, 64:128],
                stT_sb[2 * hp + 1][ro:ro + 64, j * 64:(j + 1) * 64],
                betac_sb[ro:ro + 64, hp * 10 + 5 + m:hp * 10 + 5 + m + 1], None,
                op0=TS.mult)
    if ap['_debug']:
        for hp in range(2):
            for m in range(NF):
                nc.sync.dma_start(ap['dbg_wm'][hp * NF + m], Wm[hp][m][:])

    # ---------------- Phase 4: q features + contraction + out proj --------
    outT = [persist.tile([128, S], BF16, tag=f'outT{hp}', name=f'outT{hp}')
            for hp in range(2)]
    with tc.tile_pool(name='ph4w', bufs=1) as ph4w, \
         tc.tile_pool(name='qf', bufs=2) as qfp, \
         tc.tile_pool(name='osb', bufs=3) as osbp, \
         tc.tile_pool(name='ps4', bufs=2, space='PSUM') as ps4, \
         tc.tile_pool(name='ps5', bufs=3, space='PSUM') as ps5:
        wo_sb = []
        for kc in range(2):
            w = ph4w.tile([128, D], BF16, tag=f'wo{kc}', name=f'wo{kc}')
            nc.sync.dma_start(w[:], ap['woT'][kc * 128:(kc + 1) * 128, :])
            wo_sb.append(w)
        for t in range(8):
            for hp in range(2):
                qt = qb[hp][:, t * 512:(t + 1) * 512]
                hsq = qfp.tile([128, 512], BF16, tag='qhsq', name='qhsq')
                s4t = qfp.tile([128, 512], BF16, tag='qs4t', name='qs4t')
                t5t = qfp.tile([128, 512], BF16, tag='qt5t', name='qt5t')
                qf2 = qfp.tile([128, 512], BF16, tag='qf2', name='qf2')
                qf3 = qfp.tile([128, 512], BF16, tag='qf3', name='qf3')
                qf4 = qfp.tile([128, 512], BF16, tag='qf4', name='qf4')
                qf5 = qfp.tile([128, 512], BF16, tag='qf5', name='qf5')
                nc.scalar.activation(hsq[:], qt, SQ)
                nc.vector.tensor_scalar(qf2[:], hsq[:], -0.5, None, op0=TS.add)
                nc.vector.scalar_tensor_tensor(qf3[:], hsq[:], -0.75, qt,
                                               op0=TS.add, op1=TS.mult)
                nc.scalar.activation(s4t[:], qf2[:], SQ)
                nc.vector.tensor_scalar(qf4[:], s4t[:], -0.125, None, op0=TS.add)
                nc.vector.tensor_tensor(t5t[:], qf2[:], qf3[:], op=TS.mult)
                nc.vector.scalar_tensor_tensor(qf5[:], qt, -0.0625, t5t[:],
                                               op0=TS.mult, op1=TS.add)
                rhs = [qt, qf2[:], qf3[:], qf4[:], qf5[:]]
                pO = ps4.tile([128, 512], F32, tag='pO', name='pO')
                for m in range(NF):
                    nc.tensor.matmul(pO[:], Wm[hp][m][:], rhs[m],
                                     start=(m == 0), stop=(m == NF - 1))
                dst = outT[hp][:, t * 512:(t + 1) * 512]
                if hp == 0:
                    nc.scalar.copy(dst, pO[:])
                else:
                    nc.vector.tensor_copy(dst, pO[:])
            for o in range(8):
                pP = ps5.tile([128, 512], F32, tag='pP', name='pP')
                nc.tensor.matmul(pP[:], wo_sb[0][:, o * 128:(o + 1) * 128],
                                 outT[0][:, t * 512:(t + 1) * 512],
                                 start=True, stop=False)
                nc.tensor.matmul(pP[:], wo_sb[1][:, o * 128:(o + 1) * 128],
                                 outT[1][:, t * 512:(t + 1) * 512],
                                 start=False, stop=True)
                ob = osbp.tile([128, 512], BF16, tag='ob', name='ob')
                if o % 2 == 0:
                    nc.scalar.copy(ob[:], pP[:])
                else:
                    nc.vector.tensor_copy(ob[:], pP[:])
                nc.sync.dma_start(ap['outp'][o * 128:(o + 1) * 128,
                                             t * 512:(t + 1) * 512], ob[:])

    if ap['_debug']:
        for hp in range(2):
            nc.sync.dma_start(ap['dbg_oT'][hp], outT[hp][:])


@lru_cache(maxsize=1)
def _get_program():
    return _build_program()


# ---------------------------------------------------------------------------
# Host-side packing
# ---------------------------------------------------------------------------

last_results = None


def kernel(x, w_in, w_out, beta):
    import ml_dtypes
    bf16 = ml_dtypes.bfloat16

    x = np.asarray(x, dtype=np.float32)
    w_in = np.asarray(w_in, dtype=np.float32)
    w_out = np.asarray(w_out, dtype=np.float32)
    beta = np.asarray(beta, dtype=np.float32)
    nc = _get_program()

    xT = [np.ascontiguousarray(x[b].T).astype(bf16) for b in range(B)]
    # beta' = beta_{m+1} * 4^m  (feature scales f_m = T_m / 2^(m-1))
    pow4 = (4.0 ** np.arange(NF)).astype(np.float32)
    in_maps = []
    for cid in range(NCORES):
        b, hg = cid // 4, cid % 4
        heads = [4 * hg + j for j in range(HPC)]
        wqT = np.empty((D, 256), dtype=np.float32)
        wkvT = np.empty((D, 512), dtype=np.float32)
        for hl, h in enumerate(heads):
            wqT[:, hl * 64:(hl + 1) * 64] = (SCALE * w_in[h * DH:(h + 1) * DH, :]).T
            wkvT[:, hl * 64:(hl + 1) * 64] = (SCALE * w_in[D + h * DH:D + (h + 1) * DH, :]).T
            wkvT[:, 256 + hl * 64:256 + (hl + 1) * 64] = w_in[2 * D + h * DH:2 * D + (h + 1) * DH, :].T
        woT = np.empty((256, D), dtype=np.float32)
        for hl, h in enumerate(heads):
            woT[hl * 64:(hl + 1) * 64, :] = w_out[:, h * DH:(h + 1) * DH].T
        betac = np.zeros((128, 20), dtype=np.float32)
        for hp in range(2):
            for who in range(2):
                h = heads[2 * hp + who]
                betac[:, hp * 10 + who * 5:hp * 10 + who * 5 + NF] = (
                    beta[h, 1:1 + NF] * pow4)[None, :]
        in_maps.append({
            'xT': xT[b],
            'wqT': wqT.astype(bf16),
            'wkvT': wkvT.astype(bf16),
            'woT': woT.astype(bf16),
            'betac': betac,
        })

    res = bass_utils.run_bass_kernel_spmd(nc, in_maps, core_ids=list(range(NCORES)))
    global last_results
    last_results = res

    out = np.zeros((B, S, D), dtype=np.float32)
    for cid in range(NCORES):
        out[cid // 4] += res.results[cid]['outp'].astype(np.float32).T
    return out


# revision 6
# speedup vs baseline: 4.5293x; 1.1398x over previous
"""Trainium2 Bass kernel for nn_CollapsedPBFAOptimized (Chebyshev kernelized
linear attention), bf16 fast path.

Sharding (8 cores): core c handles batch b = c//4 and the 4 heads
[4*(c%4) .. 4*(c%4)+3].  Each core computes a partial output
(x[b] @ w_in_sub -> features -> per-head KV state -> out rows) projected
through its w_out columns; the host sums the 4 partials per batch.

Math: the collapsed beta is zero for m=0 and m>=6, so the kernel is
  out_i = sum_{m=1..5} beta_m <T_m(q_i), T_m(k_j)> v_j
with only FIVE Chebyshev features per dim.  Features are computed with
scaled one-op recurrences (f1=t, f2=t^2-1/2, f3=(t^2-3/4)t, f4=f2^2-1/8,
f5=f2*f3-t/16 => f_m = T_m/2^(m-1)); the scale correction
beta_m * 4^(m-1) is folded into the block-diagonal weight tiles for the
query-side contraction.

Schedule: the per-head KV-state matmuls (phase 2) are interleaved into
phase 1's PE stream with a one-ss delay (features for a chunk are
computed on DVE/ACT while the PE works on the next ss), using an
8-chunk rotating window for k-features and v.  Query features are also
precomputed during phase 1, so the final phase is pure matmul + PSUM
eviction.  All matmuls run in bf16 (1 PE cycle/row vs 4 for fp32);
PSUM accumulates fp32.
"""
import json
import sys
import numpy as np
from contextlib import ExitStack
from functools import lru_cache

sys.path.insert(0, '/opt/trn_rl_repo')

import concourse.bass as bass
import concourse.tile as tile
from concourse import mybir, bass_utils

# ---------------------------------------------------------------------------
# Toolchain patches (walrus on this image supports one sync-wait per inst)
# ---------------------------------------------------------------------------


def _install_patches():
    from concourse.tile import ScopedClock
    from concourse import bass2jax

    def _patched_drain_and_barrier(self, tick_clock, wait_clock):
        drain_inst = self.nc.sync.drain()
        wait_clock.add_sem_waits(
            drain_inst.ins, ScopedClock({None: tick_clock.global_clock}))
        si = drain_inst.ins.sync_info
        if si is not None:
            w = list(si.on_wait)
            if len(w) > 1:
                si.on_wait = [w[0]]
                for extra in w[1:]:
                    d2 = self.nc.sync.drain()
                    d2.ins.sync_info = mybir.SyncInfo(on_wait=[extra], on_update=[])
        self.nc.all_engine_barrier()
        assert self.sems is not None
        popped = self.nc._tile_sem_poison_stack.pop()
        assert popped is self._sem_poison
        self.nc.clear_and_free_semaphores(list(self.sems.allocated().values()))
        self.nc.all_engine_barrier()

    tile.TileContext._drain_and_barrier = _patched_drain_and_barrier

    LIMIT = 1

    def split_waits_in_bir_json(bir_json):
        d = json.loads(bir_json.decode() if isinstance(bir_json, bytes) else bir_json)
        for fn in d.get('functions', []):
            for bb in fn.get('blocks', []):
                out, changed = [], False
                for ins in bb.get('instructions', []):
                    si = ins.get('sync_info')
                    waits = (si or {}).get('on_wait') or []
                    if len(waits) > LIMIT:
                        for k, w in enumerate(waits[:-LIMIT]):
                            nop = {'name': ins['name'] + f'-xw{k}',
                                   'engine': ins['engine'], 'opcode': 'NoOp',
                                   'ins': [], 'outs': [],
                                   'sync_info': {'on_wait': [w], 'on_update': []}}
                            if 'debug' in ins:
                                nop['debug'] = ins['debug']
                            out.append(nop)
                        si['on_wait'] = waits[-LIMIT:]
                        changed = True
                    out.append(ins)
                if changed:
                    bb['instructions'] = out
        return json.dumps(d).encode()

    if not getattr(bass_utils.compile_bir_kernel, '_wait_patched', False):
        orig = bass_utils.compile_bir_kernel

        def patched(bir_json, tmpdir, neff_name='file.neff'):
            return orig(split_waits_in_bir_json(bir_json), tmpdir, neff_name)

        patched._wait_patched = True
        bass_utils.compile_bir_kernel = patched
        bass2jax.compile_bir_kernel = patched


_install_patches()

# ---------------------------------------------------------------------------
# Problem constants (hardcoded per the task contract)
# ---------------------------------------------------------------------------
B, S, D = 2, 4096, 1024
H, DH = 16, 64
NF = 5                       # Chebyshev features T_1..T_5 (beta_0 = beta_{6..} = 0)
SCALE = DH ** -0.5
HPC = 4                      # heads per core
NCORES = 8
F32 = mybir.dt.float32
BF16 = mybir.dt.bfloat16
NCHUNK = S // 128            # 32 seq chunks of 128
FTC = HPC * NF * 64          # FT columns per chunk slot = 1280
NSLOT = 8                    # rotating chunk window (k features / v)

# ---------------------------------------------------------------------------
# Device program
# ---------------------------------------------------------------------------


def _build_program():
    nc = bass.Bass('TRN2', target_bir_lowering=False, debug=False,
                   num_devices=NCORES)
    ap = {}
    ap['xT'] = nc.dram_tensor('xT', (D, S), BF16, kind='ExternalInput').ap()
    ap['wqT'] = nc.dram_tensor('wqT', (D, 256), BF16, kind='ExternalInput').ap()
    ap['wkvT'] = nc.dram_tensor('wkvT', (D, 512), BF16, kind='ExternalInput').ap()
    ap['woT'] = nc.dram_tensor('woT', (256, D), BF16, kind='ExternalInput').ap()
    ap['betac'] = nc.dram_tensor('betac', (128, 20), F32, kind='ExternalInput').ap()
    ap['eye'] = nc.dram_tensor('eye', (64, 64), BF16, kind='ExternalInput').ap()
    ap['outp'] = nc.dram_tensor('outp', (D, S), BF16, kind='ExternalOutput').ap()
    import os
    ap['_debug'] = os.environ.get('KBDBG', '') == '1'
    if ap['_debug']:
        ap['dbg_qb'] = nc.dram_tensor('dbg_qb', (2, 128, S), BF16, kind='ExternalOutput').ap()
        ap['dbg_qf'] = nc.dram_tensor('dbg_qf', (8, 128, S), BF16, kind='ExternalOutput').ap()
        ap['dbg_wm'] = nc.dram_tensor('dbg_wm', (10, 128, 128), BF16, kind='ExternalOutput').ap()
        ap['dbg_oT'] = nc.dram_tensor('dbg_oT', (2, 128, S), BF16, kind='ExternalOutput').ap()

    with tile.TileContext(nc) as tc:
        with ExitStack() as ctx:
            _emit(nc, tc, ctx, ap)
    return nc


def _emit(nc, tc, ctx, ap):
    TS = mybir.AluOpType
    SQ = mybir.ActivationFunctionType.Square

    const = ctx.enter_context(tc.tile_pool(name='const', bufs=1))
    persist = ctx.enter_context(tc.tile_pool(name='persist', bufs=1))

    betac_sb = const.tile([128, 20], F32, tag='betac', name='betac')
    nc.sync.dma_start(betac_sb[:], ap['betac'][:])
    eye_sb = const.tile([64, 64], BF16, tag='eye', name='eye')
    nc.sync.dma_start(eye_sb[:], ap['eye'][:])

    qb = [persist.tile([128, S], BF16, tag=f'qb{hp}', name=f'qb{hp}') for hp in range(2)]
    # q features f2..f5 per head-pair, full length (consumed in phase 4)
    qf = [[persist.tile([128, S], BF16, tag=f'qf{hp}_{f}', name=f'qf{hp}_{f}')
           for f in range(4)] for hp in range(2)]
    FT = persist.tile([128, NSLOT * FTC], BF16, tag='FT', name='FT')
    v_all = persist.tile([128, NSLOT * 256], BF16, tag='v_all', name='v_all')
    # FT column layout: slot*1280 + hl*320 + m*64 + d   (m = 0..4 <-> T_1..T_5)
    ftv = FT[:].rearrange("p (c h m d) -> p c h m d", h=HPC, m=NF, d=64)

    # block-diagonal weight tiles for phase 4 (zeroed early; filled in ph3)
    Wm = [[persist.tile([128, 128], BF16, tag=f'wm{hp}_{m}', name=f'wm{hp}_{m}')
           for m in range(NF)] for hp in range(2)]
    for hp in range(2):
        for m in range(NF):
            nc.gpsimd.memset(Wm[hp][m][:], 0.0)

    # ---------------- Phase 1 + 2 interleaved -----------------------------
    def emit_features(pool, f1, f2, f3, f4, f5, width):
        """f1 (=clamped input) -> f2..f5 feature views; scratch from pool."""
        hsq = pool.tile([128, width], BF16, tag='hsq', name='hsq')
        s4t = pool.tile([128, width], BF16, tag='s4t', name='s4t')
        t5t = pool.tile([128, width], BF16, tag='t5t', name='t5t')
        nc.scalar.activation(hsq[:], f1, SQ)
        nc.vector.tensor_scalar(f2, hsq[:], -0.5, None, op0=TS.add)
        nc.vector.scalar_tensor_tensor(f3, hsq[:], -0.75, f1,
                                       op0=TS.add, op1=TS.mult)
        nc.scalar.activation(s4t[:], f2, SQ)
        nc.vector.tensor_scalar(f4, s4t[:], -0.125, None, op0=TS.add)
        nc.vector.tensor_tensor(t5t[:], f2, f3, op=TS.mult)
        nc.vector.scalar_tensor_tensor(f5, f1, -0.0625, t5t[:],
                                       op0=TS.mult, op1=TS.add)

    pst = None
    with tc.tile_pool(name='ph1w', bufs=1) as ph1w, \
         tc.tile_pool(name='xt', bufs=2) as xtp, \
         tc.tile_pool(name='fsc', bufs=2) as fscp, \
         tc.tile_pool(name='ps1', bufs=2, space='PSUM') as ps1, \
         tc.tile_pool(name='ps2', bufs=1, space='PSUM') as ps2:
        # k/v state accumulators: one PSUM bank per head, open all of ph1
        pst = [ps2.tile([64, 320], F32, tag=f'pst{h}', name=f'pst{h}')
               for h in range(HPC)]

        # DMA order: x(ss=0) first, then wq, then wkv (kv matmuls run after q)
        xt_next = []
        for i in range(8):
            t = xtp.tile([128, 512], BF16, tag=f'xt{i}', name=f'xt{i}')
            nc.sync.dma_start(t[:], ap['xT'][i * 128:(i + 1) * 128, 0:512])
            xt_next.append(t)
        wq_sb = []
        for i in range(8):
            wq = ph1w.tile([128, 256], BF16, tag=f'wq{i}', name=f'wq{i}')
            nc.sync.dma_start(wq[:], ap['wqT'][i * 128:(i + 1) * 128, :])
            wq_sb.append(wq)
        wkv_sb = []
        for i in range(8):
            wkv = ph1w.tile([128, 512], BF16, tag=f'wkv{i}', name=f'wkv{i}')
            nc.sync.dma_start(wkv[:], ap['wkvT'][i * 128:(i + 1) * 128, :])
            wkv_sb.append(wkv)

        def emit_p2(ss):
            # state matmuls for the 4 chunks produced by ss (features ready)
            for sc in range(4):
                c = ss * 4 + sc
                slot = c % NSLOT
                for h in range(HPC):
                    nc.tensor.matmul(
                        pst[h][:],
                        v_all[:, slot * 256 + h * 64:slot * 256 + (h + 1) * 64],
                        FT[:, slot * FTC + h * 320:slot * FTC + (h + 1) * 320],
                        start=(c == 0), stop=(c == NCHUNK - 1))

        for ss in range(8):
            xt = xt_next
            if ss < 7:
                xt_next = []
                for i in range(8):
                    t = xtp.tile([128, 512], BF16, tag=f'xt{i}', name=f'xt{i}')
                    nc.sync.dma_start(t[:], ap['xT'][i * 128:(i + 1) * 128,
                                                     (ss + 1) * 512:(ss + 2) * 512])
                    xt_next.append(t)
            for hp in range(2):
                pq = ps1.tile([128, 512], F32, tag='pq', name='pq')
                for i in range(8):
                    nc.tensor.matmul(pq[:], wq_sb[i][:, hp * 128:(hp + 1) * 128],
                                     xt[i][:], start=(i == 0), stop=(i == 7))
                nc.vector.tensor_scalar(qb[hp][:, ss * 512:(ss + 1) * 512], pq[:],
                                        -1.0, 1.0, op0=TS.max, op1=TS.min)
            for sc in range(4):
                pkv = ps1.tile([128, 512], F32, tag='pkv', name='pkv')
                for i in range(8):
                    nc.tensor.matmul(pkv[:], xt[i][:, sc * 128:(sc + 1) * 128],
                                     wkv_sb[i][:], start=(i == 0), stop=(i == 7))
                c = ss * 4 + sc
                slot = c % NSLOT
                nc.vector.tensor_scalar(ftv[:, slot, :, 0, :], pkv[:, 0:256],
                                        -1.0, 1.0, op0=TS.max, op1=TS.min)
                nc.scalar.copy(v_all[:, slot * 256:(slot + 1) * 256],
                               pkv[:, 256:512])
            # delayed phase-2 matmuls for the previous ss
            if ss > 0:
                emit_p2(ss - 1)
            # k features for this ss's chunks (2 pairs)
            for gp in range(2):
                s0 = (ss * 4 + 2 * gp) % NSLOT
                emit_features(fscp,
                              ftv[:, s0:s0 + 2, :, 0, :],
                              ftv[:, s0:s0 + 2, :, 1, :],
                              ftv[:, s0:s0 + 2, :, 2, :],
                              ftv[:, s0:s0 + 2, :, 3, :],
                              ftv[:, s0:s0 + 2, :, 4, :], 512)
            # q features for this ss
            for hp in range(2):
                sl = slice(ss * 512, (ss + 1) * 512)
                emit_features(fscp, qb[hp][:, sl],
                              qf[hp][0][:, sl], qf[hp][1][:, sl],
                              qf[hp][2][:, sl], qf[hp][3][:, sl], 512)
        emit_p2(7)
        # evict the state accumulators while ps2 is still alive
        pstsb = []
        for h in range(HPC):
            t = persist.tile([64, 320], BF16, tag=f'pstsb{h}', name=f'pstsb{h}')
            if h % 2 == 0:
                nc.scalar.copy(t[:], pst[h][:])
            else:
                nc.vector.tensor_copy(t[:], pst[h][:])
            pstsb.append(t)

    # ---------------- Phase 3: transpose state into Wm --------------------
    with tc.tile_pool(name='psT', bufs=2, space='PSUM') as psTp:
        for hp in range(2):
            for who in range(2):
                h = 2 * hp + who
                for j in range(3):
                    w = 128 if j < 2 else 64
                    pT = psTp.tile([128, 64], BF16, tag='pT', name='pT')
                    nc.tensor.transpose(pT[0:w, :],
                                        pstsb[h][:, j * 128:j * 128 + w],
                                        eye_sb[:])
                    for half in range(2):
                        m = 2 * j + half
                        if m >= NF:
                            continue
                        dst = Wm[hp][m][who * 64:(who + 1) * 64,
                                        who * 64:(who + 1) * 64]
                        src = pT[half * 64:half * 64 + 64, :]
                        sca = betac_sb[half * 64:half * 64 + 64,
                                       hp * 10 + who * 5 + m:hp * 10 + who * 5 + m + 1]
                        if who == 0:
                            nc.vector.tensor_scalar(dst, src, sca, None, op0=TS.mult)
                        else:
                            nc.scalar.activation(
                                dst, src, mybir.ActivationFunctionType.Copy,
                                bias=0.0, scale=sca)

    if ap['_debug']:
        for hp in range(2):
            nc.sync.dma_start(ap['dbg_qb'][hp], qb[hp][:])
            for f in range(4):
                nc.sync.dma_start(ap['dbg_qf'][hp * 4 + f], qf[hp][f][:])
            for m in range(NF):
                nc.sync.dma_start(ap['dbg_wm'][hp * NF + m], Wm[hp][m][:])

    # ---------------- Phase 4: q contraction + out projection -------------
    outT = [persist.tile([128, S], BF16, tag=f'outT{hp}', name=f'outT{hp}')
            for hp in range(2)]
    with tc.tile_pool(name='ph4w', bufs=1) as ph4w, \
         tc.tile_pool(name='osb', bufs=3) as osbp, \
         tc.tile_pool(name='ps4', bufs=2, space='PSUM') as ps4, \
         tc.tile_pool(name='ps5', bufs=3, space='PSUM') as ps5:
        wo_sb = []
        for kc in range(2):
            w = ph4w.tile([128, D], BF16, tag=f'wo{kc}', name=f'wo{kc}')
            nc.sync.dma_start(w[:], ap['woT'][kc * 128:(kc + 1) * 128, :])
            wo_sb.append(w)
        for t in range(8):
            sl = slice(t * 512, (t + 1) * 512)
            for hp in range(2):
                rhs = [qb[hp][:, sl]] + [qf[hp][f][:, sl] for f in range(4)]
                pO = ps4.tile([128, 512], F32, tag='pO', name='pO')
                for m in range(NF):
                    nc.tensor.matmul(pO[:], Wm[hp][m][:], rhs[m],
                                     start=(m == 0), stop=(m == NF - 1))
                dst = outT[hp][:, sl]
                if hp == 0:
                    nc.scalar.copy(dst, pO[:])
                else:
                    nc.vector.tensor_copy(dst, pO[:])
            for o in range(8):
                pP = ps5.tile([128, 512], F32, tag='pP', name='pP')
                nc.tensor.matmul(pP[:], wo_sb[0][:, o * 128:(o + 1) * 128],
                                 outT[0][:, sl], start=True, stop=False)
                nc.tensor.matmul(pP[:], wo_sb[1][:, o * 128:(o + 1) * 128],
                                 outT[1][:, sl], start=False, stop=True)
                ob = osbp.tile([128, 512], BF16, tag='ob', name='ob')
                if o % 2 == 0:
                    nc.scalar.copy(ob[:], pP[:])
                else:
                    nc.vector.tensor_copy(ob[:], pP[:])
                nc.sync.dma_start(ap['outp'][o * 128:(o + 1) * 128, sl], ob[:])

    if ap['_debug']:
        for hp in range(2):
            nc.sync.dma_start(ap['dbg_oT'][hp], outT[hp][:])


@lru_cache(maxsize=1)
def _get_program():
    return _build_program()


# ---------------------------------------------------------------------------
# Host-side packing
# ---------------------------------------------------------------------------

last_results = None


def kernel(x, w_in, w_out, beta):
    import ml_dtypes
    bf16 = ml_dtypes.bfloat16

    x = np.asarray(x, dtype=np.float32)
    w_in = np.asarray(w_in, dtype=np.float32)
    w_out = np.asarray(w_out, dtype=np.float32)
    beta = np.asarray(beta, dtype=np.float32)
    nc = _get_program()

    xT = [np.ascontiguousarray(x[b].T).astype(bf16) for b in range(B)]
    eye = np.eye(64, dtype=bf16)
    # beta' = beta_{m+1} * 4^m  (feature scales f_m = T_m / 2^(m-1))
    pow4 = (4.0 ** np.arange(NF)).astype(np.float32)
    in_maps = []
    for cid in range(NCORES):
        b, hg = cid // 4, cid % 4
        heads = [4 * hg + j for j in range(HPC)]
        wqT = np.empty((D, 256), dtype=np.float32)
        wkvT = np.empty((D, 512), dtype=np.float32)
        for hl, h in enumerate(heads):
            wqT[:, hl * 64:(hl + 1) * 64] = (SCALE * w_in[h * DH:(h + 1) * DH, :]).T
            wkvT[:, hl * 64:(hl + 1) * 64] = (SCALE * w_in[D + h * DH:D + (h + 1) * DH, :]).T
            wkvT[:, 256 + hl * 64:256 + (hl + 1) * 64] = w_in[2 * D + h * DH:2 * D + (h + 1) * DH, :].T
        woT = np.empty((256, D), dtype=np.float32)
        for hl, h in enumerate(heads):
            woT[hl * 64:(hl + 1) * 64, :] = w_out[:, h * DH:(h + 1) * DH].T
        betac = np.zeros((128, 20), dtype=np.float32)
        for hp in range(2):
            for who in range(2):
                h = heads[2 * hp + who]
                betac[:, hp * 10 + who * 5:hp * 10 + who * 5 + NF] = (
                    beta[h, 1:1 + NF] * pow4)[None, :]
        in_maps.append({
            'xT': xT[b],
            'wqT': wqT.astype(bf16),
            'wkvT': wkvT.astype(bf16),
            'woT': woT.astype(bf16),
            'betac': betac,
            'eye': eye,
        })

    res = bass_utils.run_bass_kernel_spmd(nc, in_maps, core_ids=list(range(NCORES)))
    global last_results
    last_results = res

    out = np.zeros((B, S, D), dtype=np.float32)
    for cid in range(NCORES):
        out[cid // 4] += res.results[cid]['outp'].astype(np.float32).T
    return out
